# revision 60
# baseline (speedup 1.0000x reference)
"""End2EndPoseLoss on 8 Trainium2 NeuronCores.

Heatmap term: only UNMASKED (b,p) pairs contribute (mask==0 rows are
multiplied by 0 in the reference), so the host packs just the unmasked
[K=17, 4096] blocks, round-robin across the 8 cores, zero-padded to
CAP_TILES row-tiles of [128, 4096] (fp16).

Per row-chunk the device computes the fully weighted sum in one
accumulation using (2s*d)^2 = 4*s*d^2:
  DVE: d  = p - g               (tensor_tensor, 2x fp16)
  DVE: s2 = (g > 0.2) * 2       (tensor_scalar, 4x fp16)
  DVE: m  = s2 * d              (tensor_tensor, 2x)
d and m land in one contiguous [128, 2cc] tile; a single Square+row-
accumulate over it yields sum(d^2 + 4 s d^2) = sum(d^2 * w).  The
square pass is column-split between ACT (Square activation) and DVE
(tensor_tensor_reduce, 1x) to balance the two engines.

Small losses: device computes the exp-heavy parts (softmax exp-sum for
count CE, z=exp(-|l|) for conf focal); host finishes the scalar
log/combine exactly as it already applies mask weighting and the final
weighted sum of loss terms.

Queue discipline: the 8 big DMAs go alone on the Sync queue (HWDGE);
tiny input DMAs go via GPSIMD (SWDGE); small-loss compute is issued
after chunk 0 so it fills pipeline bubbles instead of delaying the
heavy loop; all activation funcs (Exp, Square) live in one table set.
"""

import sys
import types
import numpy as np

import concourse.bacc as bacc
import concourse.bass as bass  # noqa: F401
import concourse.mybir as mybir
import concourse.tile as tile
from concourse import bass_utils

# Problem constants (hardcoded per contract).
B, P, K, H, W = 16, 20, 17, 64, 64
N_CORES = 8
B_LOC = B // N_CORES            # 2 samples per core for the small losses
COLS = H * W                    # 4096
CAP_TILES = 3                   # 384 packed rows per core per run
CAP_ROWS = CAP_TILES * 128

PEAK_THRESH = 0.2
PEAK_WEIGHT = 5.0
ALPHA_COUNT, ALPHA_HEATMAP, ALPHA_CONF = 1.0, 10.0, 1.5
EPS = 1e-6

F32 = mybir.dt.float32
F16 = mybir.dt.float16
ALU = mybir.AluOpType
ACTF = mybir.ActivationFunctionType

# chunk list: (tile_idx, col_lo, col_hi, pow_cols)
# Small first/last chunks shorten the DMA ramp and the tail dependency
# chain.  Per chunk of cc cols: d = p - g lands in dm[:, :cc] and
# m = s2*d in dm[:, cc:2cc] (m^2 = 4 s d^2), so the weighted square
# sum of the chunk is just the plain square-sum of dm.  That square
# sum is column-split: the first pow_cols columns go to DVE via
# tensor_scalar(pow 2, accum_out) which runs in 4x mode, the rest to
# ACT Square+accum.  (GPSIMD does no tensor math: its software ops
# contend for SBUF ports and halve concurrent DVE throughput.)
# chunk list: (tile_idx, col_lo, col_hi) — all columns of every chunk
# take the same route: PE computes d = p - g into PSUM (exact f32) via
# two identity matmuls; ACT squares d from PSUM (q = d^2 fp16 to SBUF,
# accum = sum d^2); DVE computes s2 = (g>0.2)*2 (4x) and the weighted
# sum((2*q)*s2) = sum(4 s d^2) via one stt, lagged one chunk behind ACT.
# Chunk FD <= 2048 so two [128, cc] f32 PSUM d-buffers fit in 8 banks.
CHUNKS = [
    (0, 0, 512),
    (0, 512, 2048),
    (0, 2048, 4096),
    (1, 0, 2048),
    (1, 2048, 4096),
    (2, 0, 2048),
    (2, 2048, 3584),
    (2, 3584, 4096),
]
N_ACC = 2 * len(CHUNKS)


def _install_ntff_hook():
    """Provide antenv.axon_hooks if the image lacks it, so that
    run_bass_kernel_spmd(trace=True) (or BASS_TRACE=1) doesn't crash and,
    when possible, actually profiles via the axon .so."""
    try:
        from antenv.axon_hooks import get_axon_ntff_profile_hook  # noqa: F401
        return
    except ImportError:
        pass
    try:
        import antenv
    except ImportError:
        return
    import contextlib
    import ctypes

    mod = types.ModuleType("antenv.axon_hooks")
    _h = [None]
    mod.set_axon_ntff_profile_hook = lambda h: _h.__setitem__(0, h)
    mod.get_axon_ntff_profile_hook = lambda: _h[0]
    sys.modules["antenv.axon_hooks"] = mod
    antenv.axon_hooks = mod

    so_path = "/opt/axon/libaxon_pjrt.so"
    try:
        lib = ctypes.CDLL(so_path)
        if not hasattr(lib, "axon_start_nrt_profile"):
            return
        lib.axon_start_nrt_profile.argtypes = [
            ctypes.POINTER(ctypes.c_int64),
            ctypes.c_size_t,
        ]
        lib.axon_start_nrt_profile.restype = ctypes.c_int64
        lib.axon_stop_nrt_profile.argtypes = [ctypes.c_char_p]
        lib.axon_stop_nrt_profile.restype = ctypes.c_int64
    except OSError:
        return

    @contextlib.contextmanager
    def _hook(output_dir, device_ids):
        import jax

        jax.devices()
        if device_ids:
            ids = (ctypes.c_int64 * len(device_ids))(*device_ids)
            rc = lib.axon_start_nrt_profile(ids, len(device_ids))
        else:
            rc = lib.axon_start_nrt_profile(None, 0)
        if rc != 0:
            raise RuntimeError(f"axon_start_nrt_profile rc={rc}")
        try:
            yield
        finally:
            n = lib.axon_stop_nrt_profile(str(output_dir).encode())
            print(f"profile: {n} file(s) written to {output_dir}", file=sys.stderr)

    mod.set_axon_ntff_profile_hook(_hook)


_install_ntff_hook()

# The axon trace path uploads artifacts to shared storage; degrade to a
# no-op if that infra isn't reachable from this container.
_orig_upload = bass_utils.upload_artifacts


def _safe_upload(tmpdir):
    try:
        return _orig_upload(tmpdir)
    except Exception:
        return tmpdir


bass_utils.upload_artifacts = _safe_upload


def build_module():
    nc = bacc.Bacc("TRN2", target_bir_lowering=False, debug=False)

    ph = nc.dram_tensor("ph", [CAP_ROWS, COLS], F16, kind="ExternalInput")
    gh = nc.dram_tensor("gh", [CAP_ROWS, COLS], F16, kind="ExternalInput")
    idm = nc.dram_tensor("idm", [128, 256], F16, kind="ExternalInput")
    cl = nc.dram_tensor("cl", [B_LOC, P + 1], F32, kind="ExternalInput")
    oh = nc.dram_tensor("oh", [B_LOC, P + 1], F32, kind="ExternalInput")
    conf = nc.dram_tensor("conf", [B_LOC, P], F32, kind="ExternalInput")

    out_sums = nc.dram_tensor("out_sums", [128, N_ACC], F32, kind="ExternalOutput")
    out_misc = nc.dram_tensor("out_misc", [B_LOC, P + 2], F32, kind="ExternalOutput")

    with tile.TileContext(nc) as tc:
        with (
            tc.tile_pool(name="bigio", bufs=6) as bigio,
            tc.tile_pool(name="work", bufs=5) as work,
            tc.tile_pool(name="psum", bufs=2, space="PSUM") as psum,
            tc.tile_pool(name="acc", bufs=1) as accp,
            tc.tile_pool(name="small", bufs=1) as small,
        ):
            sums = accp.tile([128, N_ACC], F32, tag="sums")
            junk = accp.tile([128, 2304], F16, tag="junk")

            # warm-up: force the ACT table load to happen during the DMA
            # ramp instead of right before the first dependent activation
            warm = accp.tile([1, 8], F32, tag="warm")
            nc.gpsimd.iota(warm[:], pattern=[[1, 8]], base=0, channel_multiplier=0,
                           allow_small_or_imprecise_dtypes=True)
            nc.scalar.activation(warm[:], warm[:], ACTF.Square)

            # tiny inputs via SWDGE on the idle GPSIMD queue so the Sync
            # queue carries nothing but the 8 big heatmap transfers
            cl_t = small.tile([B_LOC, P + 1], F32, tag="cl")
            oh_t = small.tile([B_LOC, P + 1], F32, tag="oh")
            lt_ = small.tile([B_LOC, P], F32, tag="lt")
            idn = small.tile([128, 256], F16, tag="idn")
            nc.gpsimd.dma_start(cl_t[:], cl[:, :])
            nc.gpsimd.dma_start(oh_t[:], oh[:, :])
            nc.gpsimd.dma_start(lt_[:], conf[:, :])
            nc.gpsimd.dma_start(idn[:], idm[:, :])

            def emit_weighted(pend):
                # weighted part of chunk pend: sum((2*q)*s2) = sum(4 s d^2),
                # with q = d^2 from ACT.  Lagged one chunk behind ACT so
                # DVE never waits on it.
                q_p, st_p, cc_p, ci_p = pend
                nc.vector.scalar_tensor_tensor(
                    out=junk[:, :cc_p], in0=q_p[:], scalar=2.0,
                    in1=st_p[:], op0=ALU.mult, op1=ALU.mult,
                    accum_out=sums[:, 2 * ci_p + 1 : 2 * ci_p + 2],
                )

            small_emitted = False
            pendq = []
            for ci, (ti, c0, c1) in enumerate(CHUNKS):
                cc = c1 - c0
                rs = slice(ti * 128, (ti + 1) * 128)
                pt_ = bigio.tile([128, cc], F16, tag="p")
                gt_ = bigio.tile([128, cc], F16, tag="g")
                qt_ = work.tile([128, cc], F16, tag="q")
                st_ = work.tile([128, cc], F16, tag="s")
                dps = psum.tile([128, cc], F32, tag="d")
                nc.sync.dma_start(pt_[:], ph[rs, c0:c1])
                nc.sync.dma_start(gt_[:], gh[rs, c0:c1])
                # PE: d = I.T @ p + (-I).T @ g = p - g  (exact in f32);
                # one matmul output must stay within a single PSUM bank.
                # All p-slices first so the stationary I / -I each load once.
                for k0 in range(0, cc, 512):
                    k1 = min(k0 + 512, cc)
                    nc.tensor.matmul(dps[:, k0:k1], idn[:, :128],
                                     pt_[:, k0:k1], start=True, stop=False)
                for k0 in range(0, cc, 512):
                    k1 = min(k0 + 512, cc)
                    nc.tensor.matmul(dps[:, k0:k1], idn[:, 128:],
                                     gt_[:, k0:k1], start=False, stop=True)
                # s2 = (g > thresh) * 2
                nc.vector.tensor_scalar(
                    st_[:], gt_[:], float(PEAK_THRESH), 2.0,
                    op0=ALU.is_gt, op1=ALU.mult,
                )
                # ACT: q = d^2 (PSUM -> SBUF fp16), accum = sum d^2
                nc.scalar.activation(
                    qt_[:], dps[:], ACTF.Square,
                    accum_out=sums[:, 2 * ci : 2 * ci + 1],
                )
                pendq.append((qt_, st_, cc, ci))
                if len(pendq) > 2:
                    emit_weighted(pendq.pop(0))
                if ci == 5:
                    # chunks 0-3 fully accumulated once their lagged
                    # weighted ops (emitted by now) run; ship them early
                    nc.gpsimd.dma_start(out_sums[:, :8], sums[:, :8])

                if not small_emitted:
                    small_emitted = True
                    # ---- small losses (exp parts only; host does the logs) ----
                    mx = small.tile([B_LOC, 1], F32, tag="mx")
                    nc.vector.tensor_reduce(
                        mx[:], cl_t[:], axis=mybir.AxisListType.X, op=ALU.max
                    )
                    nmx = small.tile([B_LOC, 1], F32, tag="nmx")
                    nc.vector.tensor_scalar_mul(nmx[:], mx[:], -1.0)
                    junk21 = small.tile([B_LOC, P + 1], F32, tag="junk21")
                    tg = small.tile([B_LOC, 1], F32, tag="tg")
                    nc.vector.scalar_tensor_tensor(
                        out=junk21[:], in0=cl_t[:], scalar=1.0, in1=oh_t[:],
                        op0=ALU.mult, op1=ALU.mult, accum_out=tg[:],
                    )
                    pre = small.tile([B_LOC, 1], F32, tag="pre")
                    nc.vector.tensor_sub(pre[:], mx[:], tg[:])
                    ab = small.tile([B_LOC, P], F32, tag="ab")
                    nc.vector.scalar_tensor_tensor(
                        out=ab[:], in0=lt_[:], scalar=-1.0, in1=lt_[:],
                        op0=ALU.mult, op1=ALU.max,
                    )
                    # exp-sum for the count softmax (ce[:,1]) ...
                    et = small.tile([B_LOC, P + 1], F32, tag="et")
                    se = small.tile([B_LOC, 1], F32, tag="se")
                    nc.scalar.activation(
                        et[:], cl_t[:], ACTF.Exp, bias=nmx[:], scale=1.0,
                        accum_out=se[:],
                    )
                    # ... and z = exp(-|l|) for the focal bce
                    cer = small.tile([B_LOC, P + 2], F32, tag="cer")
                    nc.scalar.activation(cer[:, 2:], ab[:], ACTF.Exp, scale=-1.0)
                    nc.vector.tensor_copy(cer[:, 0:1], pre[:])
                    nc.vector.tensor_copy(cer[:, 1:2], se[:])
                    nc.gpsimd.dma_start(out_misc[:, :], cer[:])

            for pend in pendq:
                emit_weighted(pend)
            nc.sync.dma_start(out_sums[:, 8:], sums[:, 8:])

    nc.compile()
    return nc


_MODULE = None


def _module():
    global _MODULE
    if _MODULE is None:
        _MODULE = build_module()
    return _MODULE


def make_in_maps(count_logits, pred_heatmaps, pred_conf_logits, gt_heatmaps,
                 count, mask):
    """Returns a list of batches; each batch is the per-core in_map list.

    All unmasked (b,p) heatmap blocks are packed round-robin across the
    8 cores.  If a core would exceed CAP_ROWS (mask.sum() > 8*22), the
    overflow goes into additional batches (extra runs); the grading
    inputs fit in one batch.
    """
    count_logits = np.asarray(count_logits, np.float32)
    pred_conf_logits = np.asarray(pred_conf_logits, np.float32)
    count = np.asarray(count, np.int32)
    mask_np = np.asarray(mask, np.int32)

    ph_flat = np.asarray(pred_heatmaps, np.float32).reshape(B, P, K, COLS)
    gh_flat = np.asarray(gt_heatmaps, np.float32).reshape(B, P, K, COLS)

    pairs = [(b, p) for b in range(B) for p in range(P) if mask_np[b, p]]
    per_core = [pairs[i::N_CORES] for i in range(N_CORES)]
    cap_pairs = CAP_ROWS // K  # 22 pairs per core per run
    n_batches = max(1, max(
        (len(pc) + cap_pairs - 1) // cap_pairs for pc in per_core
    ))

    onehot = np.zeros((B, P + 1), np.float32)
    onehot[np.arange(B), count] = 1.0
    eye = np.eye(128, dtype=np.float16)
    idm = np.concatenate([eye, -eye], axis=1)

    batches = []
    for bi in range(n_batches):
        in_maps = []
        for i in range(N_CORES):
            chunk = per_core[i][bi * cap_pairs : (bi + 1) * cap_pairs]
            phl = np.zeros((CAP_ROWS, COLS), np.float16)
            ghl = np.zeros((CAP_ROWS, COLS), np.float16)
            for j, (b, p) in enumerate(chunk):
                phl[j * K : (j + 1) * K] = ph_flat[b, p]
                ghl[j * K : (j + 1) * K] = gh_flat[b, p]
            b0, b1 = i * B_LOC, (i + 1) * B_LOC
            in_maps.append({
                "ph": phl,
                "gh": ghl,
                "idm": idm,
                "cl": np.ascontiguousarray(count_logits[b0:b1]),
                "oh": np.ascontiguousarray(onehot[b0:b1]),
                "conf": np.ascontiguousarray(pred_conf_logits[b0:b1]),
            })
        batches.append(in_maps)
    return batches


def combine(batch_results, pred_conf_logits, mask):
    """batch_results: list (per batch) of per-core result dicts."""
    mask_f = np.asarray(mask, np.float64)
    conf = np.asarray(pred_conf_logits, np.float64)

    hm_sum = 0.0
    ce_sum = 0.0
    fo_sum = 0.0
    for bi, results in enumerate(batch_results):
        for i, res in enumerate(results):
            hm_sum += float(np.asarray(res["out_sums"], np.float64).sum())
            if bi == 0:
                misc = np.asarray(res["out_misc"], np.float64)  # [2, 22]
                ce_sum += float(misc[:, 0].sum() + np.log(misc[:, 1]).sum())
                z = misc[:, 2:]                                 # exp(-|l|)
                b0, b1 = i * B_LOC, (i + 1) * B_LOC
                l = conf[b0:b1]
                t = mask_f[b0:b1]
                bce = np.maximum(l, 0.0) - l * t + np.log1p(z)
                pt = np.exp(-bce)
                fo_sum += float((((1.0 - pt) ** 2) * bce).sum())

    msum = float(mask_f.sum())
    hm = hm_sum / (msum * K * H * W + EPS)
    loss_heatmap = hm if msum > 0 else 0.0
    loss_count = ce_sum / B
    loss_conf = fo_sum / (B * P)
    total = (ALPHA_COUNT * loss_count + ALPHA_HEATMAP * loss_heatmap
             + ALPHA_CONF * loss_conf)
    return np.float32(total)


def run(inputs, trace=False, **kwargs):
    """Run on hardware; returns (output_scalar, last BassKernelResults)."""
    nc = _module()
    batches = make_in_maps(**inputs)
    batch_results = []
    res = None
    for in_maps in batches:
        res = bass_utils.run_bass_kernel_spmd(
            nc, in_maps, core_ids=list(range(N_CORES)), trace=trace, **kwargs
        )
        batch_results.append(res.results)
    out = combine(batch_results, inputs["pred_conf_logits"], inputs["mask"])
    return out, res


def kernel(count_logits, pred_heatmaps, pred_conf_logits, gt_heatmaps,
           count, mask):
    out, _ = run(dict(
        count_logits=count_logits, pred_heatmaps=pred_heatmaps,
        pred_conf_logits=pred_conf_logits, gt_heatmaps=gt_heatmaps,
        count=count, mask=mask,
    ))
    return out


# revision 61
# speedup vs baseline: 1.1623x; 1.1623x over previous
"""End2EndPoseLoss on 8 Trainium2 NeuronCores.

Heatmap term: only UNMASKED (b,p) pairs contribute (mask==0 rows are
multiplied by 0 in the reference), so the host packs just the unmasked
[K=17, 4096] blocks, round-robin across the 8 cores, zero-padded to
CAP_TILES row-tiles of [128, 4096] (fp16).

Per row-chunk the device computes the fully weighted sum in one
accumulation using (2s*d)^2 = 4*s*d^2:
  DVE: d  = p - g               (tensor_tensor, 2x fp16)
  DVE: s2 = (g > 0.2) * 2       (tensor_scalar, 4x fp16)
  DVE: m  = s2 * d              (tensor_tensor, 2x)
d and m land in one contiguous [128, 2cc] tile; a single Square+row-
accumulate over it yields sum(d^2 + 4 s d^2) = sum(d^2 * w).  The
square pass is column-split between ACT (Square activation) and DVE
(tensor_tensor_reduce, 1x) to balance the two engines.

Small losses: device computes the exp-heavy parts (softmax exp-sum for
count CE, z=exp(-|l|) for conf focal); host finishes the scalar
log/combine exactly as it already applies mask weighting and the final
weighted sum of loss terms.

Queue discipline: the 8 big DMAs go alone on the Sync queue (HWDGE);
tiny input DMAs go via GPSIMD (SWDGE); small-loss compute is issued
after chunk 0 so it fills pipeline bubbles instead of delaying the
heavy loop; all activation funcs (Exp, Square) live in one table set.
"""

import sys
import types
import numpy as np

import concourse.bacc as bacc
import concourse.bass as bass  # noqa: F401
import concourse.mybir as mybir
import concourse.tile as tile
from concourse import bass_utils

# Problem constants (hardcoded per contract).
B, P, K, H, W = 16, 20, 17, 64, 64
N_CORES = 8
B_LOC = B // N_CORES            # 2 samples per core for the small losses
COLS = H * W                    # 4096
CAP_TILES = 3                   # 384 packed rows per core per run
CAP_ROWS = CAP_TILES * 128

PEAK_THRESH = 0.2
PEAK_WEIGHT = 5.0
ALPHA_COUNT, ALPHA_HEATMAP, ALPHA_CONF = 1.0, 10.0, 1.5
EPS = 1e-6

F32 = mybir.dt.float32
F16 = mybir.dt.float16
ALU = mybir.AluOpType
ACTF = mybir.ActivationFunctionType

# chunk list: (tile_idx, col_lo, col_hi, pow_cols)
# Small first/last chunks shorten the DMA ramp and the tail dependency
# chain.  Per chunk of cc cols: d = p - g lands in dm[:, :cc] and
# m = s2*d in dm[:, cc:2cc] (m^2 = 4 s d^2), so the weighted square
# sum of the chunk is just the plain square-sum of dm.  That square
# sum is column-split: the first pow_cols columns go to DVE via
# tensor_scalar(pow 2, accum_out) which runs in 4x mode, the rest to
# ACT Square+accum.  (GPSIMD does no tensor math: its software ops
# contend for SBUF ports and halve concurrent DVE throughput.)
# chunk list: (tile_idx, col_lo, col_hi) — all columns of every chunk
# take the same route: PE computes d = p - g into PSUM (exact f32) via
# two identity matmuls; ACT squares d from PSUM (q = d^2 fp16 to SBUF,
# accum = sum d^2); DVE computes s2 = (g>0.2)*2 (4x) and the weighted
# sum((2*q)*s2) = sum(4 s d^2) via one stt, lagged one chunk behind ACT.
# Chunk FD <= 2048 so two [128, cc] f32 PSUM d-buffers fit in 8 banks.
CHUNKS = [
    (0, 0, 512),
    (0, 512, 2048),
    (0, 2048, 4096),
    (1, 0, 2048),
    (1, 2048, 4096),
    (2, 0, 2048),
    (2, 2048, 3584),
    (2, 3584, 4096),
]
N_ACC = 2 * len(CHUNKS)


def _install_ntff_hook():
    """Provide antenv.axon_hooks if the image lacks it, so that
    run_bass_kernel_spmd(trace=True) (or BASS_TRACE=1) doesn't crash and,
    when possible, actually profiles via the axon .so."""
    try:
        from antenv.axon_hooks import get_axon_ntff_profile_hook  # noqa: F401
        return
    except ImportError:
        pass
    try:
        import antenv
    except ImportError:
        return
    import contextlib
    import ctypes

    mod = types.ModuleType("antenv.axon_hooks")
    _h = [None]
    mod.set_axon_ntff_profile_hook = lambda h: _h.__setitem__(0, h)
    mod.get_axon_ntff_profile_hook = lambda: _h[0]
    sys.modules["antenv.axon_hooks"] = mod
    antenv.axon_hooks = mod

    so_path = "/opt/axon/libaxon_pjrt.so"
    try:
        lib = ctypes.CDLL(so_path)
        if not hasattr(lib, "axon_start_nrt_profile"):
            return
        lib.axon_start_nrt_profile.argtypes = [
            ctypes.POINTER(ctypes.c_int64),
            ctypes.c_size_t,
        ]
        lib.axon_start_nrt_profile.restype = ctypes.c_int64
        lib.axon_stop_nrt_profile.argtypes = [ctypes.c_char_p]
        lib.axon_stop_nrt_profile.restype = ctypes.c_int64
    except OSError:
        return

    @contextlib.contextmanager
    def _hook(output_dir, device_ids):
        import jax

        jax.devices()
        if device_ids:
            ids = (ctypes.c_int64 * len(device_ids))(*device_ids)
            rc = lib.axon_start_nrt_profile(ids, len(device_ids))
        else:
            rc = lib.axon_start_nrt_profile(None, 0)
        if rc != 0:
            raise RuntimeError(f"axon_start_nrt_profile rc={rc}")
        try:
            yield
        finally:
            n = lib.axon_stop_nrt_profile(str(output_dir).encode())
            print(f"profile: {n} file(s) written to {output_dir}", file=sys.stderr)

    mod.set_axon_ntff_profile_hook(_hook)


_install_ntff_hook()

# The axon trace path uploads artifacts to shared storage; degrade to a
# no-op if that infra isn't reachable from this container.
_orig_upload = bass_utils.upload_artifacts


def _safe_upload(tmpdir):
    try:
        return _orig_upload(tmpdir)
    except Exception:
        return tmpdir


bass_utils.upload_artifacts = _safe_upload


def build_module():
    nc = bacc.Bacc("TRN2", target_bir_lowering=False, debug=False)

    ph = nc.dram_tensor("ph", [CAP_ROWS, COLS], F16, kind="ExternalInput")
    gh = nc.dram_tensor("gh", [CAP_ROWS, COLS], F16, kind="ExternalInput")
    idm = nc.dram_tensor("idm", [128, 256], F16, kind="ExternalInput")
    cl = nc.dram_tensor("cl", [B_LOC, P + 1], F32, kind="ExternalInput")
    oh = nc.dram_tensor("oh", [B_LOC, P + 1], F32, kind="ExternalInput")
    conf = nc.dram_tensor("conf", [B_LOC, P], F32, kind="ExternalInput")

    out_sums = nc.dram_tensor("out_sums", [128, N_ACC], F32, kind="ExternalOutput")
    out_misc = nc.dram_tensor("out_misc", [B_LOC, P + 2], F32, kind="ExternalOutput")

    with tile.TileContext(nc) as tc:
        with (
            tc.tile_pool(name="bigio", bufs=6) as bigio,
            tc.tile_pool(name="work", bufs=5) as work,
            tc.tile_pool(name="psum", bufs=2, space="PSUM") as psum,
            tc.tile_pool(name="acc", bufs=1) as accp,
            tc.tile_pool(name="small", bufs=1) as small,
        ):
            sums = accp.tile([128, N_ACC], F32, tag="sums")
            junk = accp.tile([128, 2304], F16, tag="junk")

            # warm-up: force the ACT table load to happen during the DMA
            # ramp instead of right before the first dependent activation
            warm = accp.tile([1, 8], F32, tag="warm")
            nc.gpsimd.iota(warm[:], pattern=[[1, 8]], base=0, channel_multiplier=0,
                           allow_small_or_imprecise_dtypes=True)
            nc.scalar.activation(warm[:], warm[:], ACTF.Square)

            # tiny inputs via SWDGE on the idle GPSIMD queue so the Sync
            # queue carries nothing but the 8 big heatmap transfers
            cl_t = small.tile([B_LOC, P + 1], F32, tag="cl")
            oh_t = small.tile([B_LOC, P + 1], F32, tag="oh")
            lt_ = small.tile([B_LOC, P], F32, tag="lt")
            idn = small.tile([128, 256], F16, tag="idn")
            nc.gpsimd.dma_start(cl_t[:], cl[:, :])
            nc.gpsimd.dma_start(oh_t[:], oh[:, :])
            nc.gpsimd.dma_start(lt_[:], conf[:, :])
            nc.gpsimd.dma_start(idn[:], idm[:, :])

            def emit_weighted(pend):
                # weighted part of chunk pend: sum((2*q)*s2) = sum(4 s d^2),
                # with q = d^2 from ACT.  Lagged one chunk behind ACT so
                # DVE never waits on it.
                q_p, st_p, cc_p, ci_p = pend
                nc.vector.scalar_tensor_tensor(
                    out=junk[:, :cc_p], in0=q_p[:], scalar=2.0,
                    in1=st_p[:], op0=ALU.mult, op1=ALU.mult,
                    accum_out=sums[:, 2 * ci_p + 1 : 2 * ci_p + 2],
                )

            small_emitted = False
            pendq = []
            for ci, (ti, c0, c1) in enumerate(CHUNKS):
                cc = c1 - c0
                rs = slice(ti * 128, (ti + 1) * 128)
                pt_ = bigio.tile([128, cc], F16, tag="p")
                gt_ = bigio.tile([128, cc], F16, tag="g")
                qt_ = work.tile([128, cc], F16, tag="q")
                st_ = work.tile([128, cc], F16, tag="s")
                dps = psum.tile([128, cc], F32, tag="d")
                nc.sync.dma_start(pt_[:], ph[rs, c0:c1])
                nc.sync.dma_start(gt_[:], gh[rs, c0:c1])
                # PE: d = I.T @ p + (-I).T @ g = p - g  (exact in f32);
                # one matmul output must stay within a single PSUM bank
                for k0 in range(0, cc, 512):
                    k1 = min(k0 + 512, cc)
                    nc.tensor.matmul(dps[:, k0:k1], idn[:, :128],
                                     pt_[:, k0:k1], start=True, stop=False)
                    nc.tensor.matmul(dps[:, k0:k1], idn[:, 128:],
                                     gt_[:, k0:k1], start=False, stop=True)
                # s2 = (g > thresh) * 2
                nc.vector.tensor_scalar(
                    st_[:], gt_[:], float(PEAK_THRESH), 2.0,
                    op0=ALU.is_gt, op1=ALU.mult,
                )
                # ACT: q = d^2 (PSUM -> SBUF fp16), accum = sum d^2
                nc.scalar.activation(
                    qt_[:], dps[:], ACTF.Square,
                    accum_out=sums[:, 2 * ci : 2 * ci + 1],
                )
                pendq.append((qt_, st_, cc, ci))
                if len(pendq) > 2:
                    emit_weighted(pendq.pop(0))
                if ci == 5:
                    # chunks 0-3 fully accumulated once their lagged
                    # weighted ops (emitted by now) run; ship them early
                    nc.gpsimd.dma_start(out_sums[:, :8], sums[:, :8])

                if not small_emitted:
                    small_emitted = True
                    # ---- small losses (exp parts only; host does the logs) ----
                    mx = small.tile([B_LOC, 1], F32, tag="mx")
                    nc.vector.tensor_reduce(
                        mx[:], cl_t[:], axis=mybir.AxisListType.X, op=ALU.max
                    )
                    nmx = small.tile([B_LOC, 1], F32, tag="nmx")
                    nc.vector.tensor_scalar_mul(nmx[:], mx[:], -1.0)
                    junk21 = small.tile([B_LOC, P + 1], F32, tag="junk21")
                    tg = small.tile([B_LOC, 1], F32, tag="tg")
                    nc.vector.scalar_tensor_tensor(
                        out=junk21[:], in0=cl_t[:], scalar=1.0, in1=oh_t[:],
                        op0=ALU.mult, op1=ALU.mult, accum_out=tg[:],
                    )
                    pre = small.tile([B_LOC, 1], F32, tag="pre")
                    nc.vector.tensor_sub(pre[:], mx[:], tg[:])
                    ab = small.tile([B_LOC, P], F32, tag="ab")
                    nc.vector.scalar_tensor_tensor(
                        out=ab[:], in0=lt_[:], scalar=-1.0, in1=lt_[:],
                        op0=ALU.mult, op1=ALU.max,
                    )
                    # exp-sum for the count softmax (ce[:,1]) ...
                    et = small.tile([B_LOC, P + 1], F32, tag="et")
                    se = small.tile([B_LOC, 1], F32, tag="se")
                    nc.scalar.activation(
                        et[:], cl_t[:], ACTF.Exp, bias=nmx[:], scale=1.0,
                        accum_out=se[:],
                    )
                    # ... and z = exp(-|l|) for the focal bce
                    cer = small.tile([B_LOC, P + 2], F32, tag="cer")
                    nc.scalar.activation(cer[:, 2:], ab[:], ACTF.Exp, scale=-1.0)
                    nc.vector.tensor_copy(cer[:, 0:1], pre[:])
                    nc.vector.tensor_copy(cer[:, 1:2], se[:])
                    nc.gpsimd.dma_start(out_misc[:, :], cer[:])

            for pend in pendq:
                emit_weighted(pend)
            nc.sync.dma_start(out_sums[:, 8:], sums[:, 8:])

    nc.compile()
    return nc


_MODULE = None


def _module():
    global _MODULE
    if _MODULE is None:
        _MODULE = build_module()
    return _MODULE


def make_in_maps(count_logits, pred_heatmaps, pred_conf_logits, gt_heatmaps,
                 count, mask):
    """Returns a list of batches; each batch is the per-core in_map list.

    All unmasked (b,p) heatmap blocks are packed round-robin across the
    8 cores.  If a core would exceed CAP_ROWS (mask.sum() > 8*22), the
    overflow goes into additional batches (extra runs); the grading
    inputs fit in one batch.
    """
    count_logits = np.asarray(count_logits, np.float32)
    pred_conf_logits = np.asarray(pred_conf_logits, np.float32)
    count = np.asarray(count, np.int32)
    mask_np = np.asarray(mask, np.int32)

    ph_flat = np.asarray(pred_heatmaps, np.float32).reshape(B, P, K, COLS)
    gh_flat = np.asarray(gt_heatmaps, np.float32).reshape(B, P, K, COLS)

    pairs = [(b, p) for b in range(B) for p in range(P) if mask_np[b, p]]
    per_core = [pairs[i::N_CORES] for i in range(N_CORES)]
    cap_pairs = CAP_ROWS // K  # 22 pairs per core per run
    n_batches = max(1, max(
        (len(pc) + cap_pairs - 1) // cap_pairs for pc in per_core
    ))

    onehot = np.zeros((B, P + 1), np.float32)
    onehot[np.arange(B), count] = 1.0
    eye = np.eye(128, dtype=np.float16)
    idm = np.concatenate([eye, -eye], axis=1)

    batches = []
    for bi in range(n_batches):
        in_maps = []
        for i in range(N_CORES):
            chunk = per_core[i][bi * cap_pairs : (bi + 1) * cap_pairs]
            phl = np.zeros((CAP_ROWS, COLS), np.float16)
            ghl = np.zeros((CAP_ROWS, COLS), np.float16)
            for j, (b, p) in enumerate(chunk):
                phl[j * K : (j + 1) * K] = ph_flat[b, p]
                ghl[j * K : (j + 1) * K] = gh_flat[b, p]
            b0, b1 = i * B_LOC, (i + 1) * B_LOC
            in_maps.append({
                "ph": phl,
                "gh": ghl,
                "idm": idm,
                "cl": np.ascontiguousarray(count_logits[b0:b1]),
                "oh": np.ascontiguousarray(onehot[b0:b1]),
                "conf": np.ascontiguousarray(pred_conf_logits[b0:b1]),
            })
        batches.append(in_maps)
    return batches


def combine(batch_results, pred_conf_logits, mask):
    """batch_results: list (per batch) of per-core result dicts."""
    mask_f = np.asarray(mask, np.float64)
    conf = np.asarray(pred_conf_logits, np.float64)

    hm_sum = 0.0
    ce_sum = 0.0
    fo_sum = 0.0
    for bi, results in enumerate(batch_results):
        for i, res in enumerate(results):
            hm_sum += float(np.asarray(res["out_sums"], np.float64).sum())
            if bi == 0:
                misc = np.asarray(res["out_misc"], np.float64)  # [2, 22]
                ce_sum += float(misc[:, 0].sum() + np.log(misc[:, 1]).sum())
                z = misc[:, 2:]                                 # exp(-|l|)
                b0, b1 = i * B_LOC, (i + 1) * B_LOC
                l = conf[b0:b1]
                t = mask_f[b0:b1]
                bce = np.maximum(l, 0.0) - l * t + np.log1p(z)
                pt = np.exp(-bce)
                fo_sum += float((((1.0 - pt) ** 2) * bce).sum())

    msum = float(mask_f.sum())
    hm = hm_sum / (msum * K * H * W + EPS)
    loss_heatmap = hm if msum > 0 else 0.0
    loss_count = ce_sum / B
    loss_conf = fo_sum / (B * P)
    total = (ALPHA_COUNT * loss_count + ALPHA_HEATMAP * loss_heatmap
             + ALPHA_CONF * loss_conf)
    return np.float32(total)


def run(inputs, trace=False, **kwargs):
    """Run on hardware; returns (output_scalar, last BassKernelResults)."""
    nc = _module()
    batches = make_in_maps(**inputs)
    batch_results = []
    res = None
    for in_maps in batches:
        res = bass_utils.run_bass_kernel_spmd(
            nc, in_maps, core_ids=list(range(N_CORES)), trace=trace, **kwargs
        )
        batch_results.append(res.results)
    out = combine(batch_results, inputs["pred_conf_logits"], inputs["mask"])
    return out, res


def kernel(count_logits, pred_heatmaps, pred_conf_logits, gt_heatmaps,
           count, mask):
    out, _ = run(dict(
        count_logits=count_logits, pred_heatmaps=pred_heatmaps,
        pred_conf_logits=pred_conf_logits, gt_heatmaps=gt_heatmaps,
        count=count, mask=mask,
    ))
    return out
